# revision 22
# baseline (speedup 1.0000x reference)
"""Multi-head attention (B=2, S=2048, D=1024, H=16) on 8 Trainium2 cores.

Sharding: tensor-parallel over heads for QKV+attention (2 heads/core),
then an AllToAll reshards the attention output so each core computes the
output projection for its own 256-row slice of the sequence (both batches).
Host assembles the full output by concatenating the per-core slices.

Device pipeline per core (SPMD, identical program, shard-specific data):
  - starting-gun AllGather to absorb inter-core launch skew
  - PE warmup matmuls while x^T streams in (HAM clock ramp)
  - QKV projection (bf16): batch 0 in a ch-outer fast-start variant,
    batch 1 interleaved into batch-0 attention emission (PE executes its
    instruction stream in order)
  - attention: S^T = K^T.T @ Q^T, exp on ACT (PSUM [128,1024]), A*V with a
    ones column on V producing the softmax denominator (M=65), normalize
    via reciprocal on a [64,8] reshape + DMA partition-broadcast
  - AllToAll (bf16) per batch, out-projection overlapped.

PSUM budget (8 banks): tag s [128,1024]x2 = 4, tag av [65,512]x3 = 3,
tag o [128,512]x1 = 1. QK proj borrows s slots, V/warmup/outproj borrow o.
"""
import os
import sys

sys.path.insert(0, "/opt/trn_rl_repo")

import numpy as np
import ml_dtypes

import concourse.bass as bass
import concourse.tile as tile
from concourse import bacc, mybir
from concourse import bass_utils

B = 2
S = 2048
D = 1024
H = 16
DH = 64
N_CORES = 8
HEADS_PER_CORE = H // N_CORES          # 2
S_SLICE = S // N_CORES                 # 256
N_CH = D // 128                        # 8 contraction chunks
N_QT = S // 512                        # 4 q tiles
N_KC = S // 128                        # 16 k chunks

F32 = mybir.dt.float32
BF16 = mybir.dt.bfloat16

_compiled = None
last_results = None


def _build():
    nc = bacc.Bacc(
        "TRN2",
        target_bir_lowering=False,
        debug=False,
        enable_asserts=True,
        num_devices=N_CORES,
    )

    xtb = nc.dram_tensor("xtb", [B, 128, N_CH, S], BF16, kind="ExternalInput").ap()
    wqt = nc.dram_tensor("wqt", [128, N_CH, 128], BF16, kind="ExternalInput").ap()
    wkt = nc.dram_tensor("wkt", [128, N_CH, 128], BF16, kind="ExternalInput").ap()
    wvt = nc.dram_tensor("wvt", [128, N_CH, 128], BF16, kind="ExternalInput").ap()
    wot = nc.dram_tensor("wot", [128, N_CH, D], BF16, kind="ExternalInput").ap()
    bb = nc.dram_tensor("bb", [128, D], F32, kind="ExternalInput").ap()
    oc = nc.dram_tensor("oc", [B, S_SLICE, D], F32, kind="ExternalOutput").ap()

    EXP = mybir.ActivationFunctionType.Exp
    SCALE = DH ** -0.5

    with tile.TileContext(nc) as tc:
        with (
            tc.tile_pool(name="w", bufs=1) as wp,
            tc.tile_pool(name="qkt", bufs=1) as qktp,
            tc.tile_pool(name="vsb", bufs=1) as vsbp,
            tc.tile_pool(name="xtb", bufs=2) as xtbp,
            tc.tile_pool(name="pt", bufs=2) as ptp,
            tc.tile_pool(name="norm", bufs=2) as normp,
            tc.tile_pool(name="x2", bufs=1) as x2p,
            tc.tile_pool(name="outsb", bufs=2) as outp,
            tc.tile_pool(name="dram", bufs=1, space="DRAM") as dram,
            tc.tile_pool(name="dramsc", bufs=4, space="DRAM") as dramsc,
            tc.tile_pool(name="sps", bufs=2, space="PSUM") as sps,
            tc.tile_pool(name="avps", bufs=3, space="PSUM") as avps,
            tc.tile_pool(name="ops", bufs=1, space="PSUM") as ops,
        ):
            # ---- weights ----
            wqt_sb = wp.tile([128, N_CH * 128], BF16)
            nc.sync.dma_start(wqt_sb[:], wqt[:].rearrange("p c e -> p (c e)"))
            wkt_sb = wp.tile([128, N_CH * 128], BF16)
            nc.sync.dma_start(wkt_sb[:], wkt[:].rearrange("p c e -> p (c e)"))
            wvt_sb = wp.tile([128, N_CH * 128], BF16)
            nc.sync.dma_start(wvt_sb[:], wvt[:].rearrange("p c e -> p (c e)"))

            # ---- starting gun: tiny AllGather aligns the 8 cores ----
            gun_in = dram.tile([1, 16], F32, name="gun_in")
            gun_out = dram.tile([N_CORES, 16], F32, name="gun_out")
            gun_sb = wp.tile([1, 16], F32)
            nc.gpsimd.memset(gun_sb[:], 0.0)
            nc.sync.dma_start(gun_in[:], gun_sb[:])
            nc.gpsimd.collective_compute(
                "AllGather", mybir.AluOpType.bypass,
                replica_groups=[list(range(N_CORES))],
                ins=[gun_in[:]], outs=[gun_out[:]],
            )

            # ---- PE warmup while DMAs stream ----
            warm = wp.tile([128, 512], BF16)
            nc.gpsimd.memset(warm[:], 0.0)
            for i in range(48):
                wps = ops.tile([128, 512], F32, tag="o", name="wps")
                nc.tensor.matmul(wps[:], lhsT=warm[:, 0:128], rhs=warm[:],
                                 start=True, stop=True)

            xtb_sbs = [None, None]
            Qt, Kt, Vs = [], [], []
            for b in range(B):
                Qt.append(qktp.tile([128, S], BF16, tag=f"qt{b}", name=f"qt{b}"))
                Kt.append(qktp.tile([128, S], BF16, tag=f"kt{b}", name=f"kt{b}"))
                Vs.append(vsbp.tile([128, N_KC * 130], BF16, tag=f"v{b}",
                                    name=f"v{b}"))

            def emit_xtb_load(b):
                t_ = xtbp.tile([128, N_CH * S], BF16, tag="xtb", name="xtb_sb")
                xtb_sbs[b] = t_
                for ch in range(N_CH):
                    nc.sync.dma_start(t_[:, ch * S:(ch + 1) * S], xtb[b, :, ch, :])

            def emit_qk_fast(b):
                """ch-outer, 2 passes of 2 t-tiles; starts as chunks land."""
                for p_ in range(2):
                    q_ps = sps.tile([128, 1024], F32, tag="s", name="q_ps")
                    k_ps = sps.tile([128, 1024], F32, tag="s", name="k_ps")
                    for ch in range(N_CH):
                        for j in range(2):
                            t = 2 * p_ + j
                            rhs = xtb_sbs[b][:, ch * S + t * 512:
                                             ch * S + (t + 1) * 512]
                            nc.tensor.matmul(
                                q_ps[:, j * 512:(j + 1) * 512],
                                lhsT=wqt_sb[:, ch * 128:(ch + 1) * 128],
                                rhs=rhs, start=(ch == 0), stop=(ch == N_CH - 1),
                            )
                        for j in range(2):
                            t = 2 * p_ + j
                            rhs = xtb_sbs[b][:, ch * S + t * 512:
                                             ch * S + (t + 1) * 512]
                            nc.tensor.matmul(
                                k_ps[:, j * 512:(j + 1) * 512],
                                lhsT=wkt_sb[:, ch * 128:(ch + 1) * 128],
                                rhs=rhs, start=(ch == 0), stop=(ch == N_CH - 1),
                            )
                    nc.vector.tensor_copy(
                        Qt[b][:, p_ * 1024:(p_ + 1) * 1024], q_ps[:])
                    nc.vector.tensor_copy(
                        Kt[b][:, p_ * 1024:(p_ + 1) * 1024], k_ps[:])

            def emit_qk_slice(b, t):
                """ch-inner, one psum at a time (attention running)."""
                for which, w_sb, dst in (("q", wqt_sb, Qt[b]), ("k", wkt_sb, Kt[b])):
                    ps_ = sps.tile([128, 1024], F32, tag="s", name=f"{which}_ps1")
                    for ch in range(N_CH):
                        nc.tensor.matmul(
                            ps_[:, 0:512],
                            lhsT=w_sb[:, ch * 128:(ch + 1) * 128],
                            rhs=xtb_sbs[b][:, ch * S + t * 512:
                                           ch * S + (t + 1) * 512],
                            start=(ch == 0), stop=(ch == N_CH - 1),
                        )
                    nc.vector.tensor_copy(
                        dst[:, t * 512:(t + 1) * 512], ps_[:, 0:512])

            def emit_v(b, sts):
                v_sb = Vs[b]
                for st in sts:
                    v_ps = ops.tile([128, 512], F32, tag="o", name="v_ps")
                    for ch in range(N_CH):
                        nc.tensor.matmul(
                            v_ps[:, 0:128],
                            lhsT=xtb_sbs[b][:, ch * S + st * 128:
                                            ch * S + (st + 1) * 128],
                            rhs=wvt_sb[:, ch * 128:(ch + 1) * 128],
                            start=(ch == 0), stop=(ch == N_CH - 1),
                        )
                    dst = v_sb[:].rearrange("p (c o) -> p c o", o=65)[
                        :, 2 * st:2 * st + 2, 0:64
                    ]
                    nc.vector.tensor_copy(
                        dst, v_ps[:, 0:128].rearrange("p (h e) -> p h e", e=64)
                    )

            a2a_in = [dram.tile([N_CORES, 128, S_SLICE], BF16, tag=f"a2ai{b}",
                                name=f"a2ai{b}") for b in range(B)]
            a2a_out = [dram.tile([N_CORES, 128, S_SLICE], BF16, tag=f"a2ao{b}",
                                 name=f"a2ao{b}") for b in range(B)]

            def att_unit(b, h, t, v_interleave=None):
                hp = slice(h * 64, (h + 1) * 64)
                voff = h * 65
                av = avps.tile([65, 512], F32, tag="av", name="av")
                qs = slice(t * 512, (t + 1) * 512)
                for cc in range(N_KC // 2):
                    if v_interleave is not None:
                        emit_v(v_interleave, (2 * cc, 2 * cc + 1))
                    s_ps = sps.tile([128, 1024], F32, tag="s", name="s_ps")
                    for j in range(2):
                        c = 2 * cc + j
                        ks = slice(c * 128, (c + 1) * 128)
                        nc.tensor.matmul(
                            s_ps[:, j * 512:(j + 1) * 512],
                            lhsT=Kt[b][hp, ks], rhs=Qt[b][hp, qs],
                            start=True, stop=True,
                        )
                    p_sb = ptp.tile([128, 1024], BF16, tag="p", name="p_sb")
                    nc.scalar.activation(p_sb[:], s_ps[:], EXP, scale=SCALE)
                    for j in range(2):
                        c = 2 * cc + j
                        nc.tensor.matmul(
                            av[:],
                            lhsT=Vs[b][:, c * 130 + voff:c * 130 + voff + 65],
                            rhs=p_sb[:, j * 512:(j + 1) * 512],
                            start=(c == 0), stop=(c == N_KC - 1),
                            skip_group_check=True,
                        )
                # normalize: denom -> [64,8] reshape -> reciprocal -> broadcast
                den_sb = normp.tile([1, 512], F32, tag="dsb", name="den_sb")
                nc.vector.tensor_copy(den_sb[:], av[64:65, :])
                den_d = dramsc.tile([512], F32, tag="dend", name="den_d")
                nc.sync.dma_start(
                    den_d[:].rearrange("(a q) -> a q", a=1), den_sb[:])
                den64 = normp.tile([64, 8], F32, tag="d64", name="den64")
                nc.sync.dma_start(
                    den64[:], den_d[:].rearrange("(p q) -> p q", p=64))
                rec64 = normp.tile([64, 8], F32, tag="r64", name="rec64")
                nc.vector.reciprocal(rec64[:], den64[:])
                rsc = dramsc.tile([512], F32, tag="rsc", name="rsc")
                nc.sync.dma_start(
                    rsc[:].rearrange("(p q) -> p q", p=64), rec64[:])
                bcast = normp.tile([64, 512], F32, tag="bc", name="bcast")
                nc.sync.dma_start(
                    bcast[:],
                    rsc[:].rearrange("(a q) -> a q", a=1).broadcast_to([64, 512]),
                )
                o_sb = normp.tile([64, 512], BF16, tag="ob", name="o_sb")
                nc.vector.tensor_mul(o_sb[:], av[0:64, :], bcast[:])
                for j in range(2):
                    nc.sync.dma_start(
                        a2a_in[b][2 * t + j, hp, :],
                        o_sb[:, j * S_SLICE:(j + 1) * S_SLICE],
                    )

            def emit_a2a(b):
                nc.gpsimd.collective_compute(
                    "AllToAll", mybir.AluOpType.bypass,
                    replica_groups=[list(range(N_CORES))],
                    ins=[a2a_in[b][:]], outs=[a2a_out[b][:]],
                )

            x2_tiles = {}

            def emit_x2_loads(b):
                x2 = []
                for i in range(N_CH):
                    x2_sb = x2p.tile([128, S_SLICE], BF16, tag=f"x2_{b}_{i}",
                                     name=f"x2_{b}_{i}")
                    nc.sync.dma_start(x2_sb[:], a2a_out[b][i])
                    x2.append(x2_sb)
                x2_tiles[b] = x2

            def emit_outproj_piece(b, st, et, wot_sb, bb_sb):
                o_ps = ops.tile([128, 512], F32, tag="o", name="o_ps")
                for ch in range(N_CH):
                    nc.tensor.matmul(
                        o_ps[:],
                        lhsT=x2_tiles[b][ch][:, st * 128:(st + 1) * 128],
                        rhs=wot_sb[:, ch * D + et * 512:ch * D + (et + 1) * 512],
                        start=(ch == 0), stop=(ch == N_CH - 1),
                    )
                out_sb = outp.tile([128, 512], F32, tag="osb", name="out_sb")
                nc.vector.tensor_add(
                    out_sb[:], o_ps[:], bb_sb[:, et * 512:(et + 1) * 512])
                nc.sync.dma_start(
                    oc[b, st * 128:(st + 1) * 128, et * 512:(et + 1) * 512],
                    out_sb[:],
                )

            # ================= pipeline =================
            wot_sb = wp.tile([128, N_CH * D], BF16)
            nc.sync.dma_start(wot_sb[:], wot[:].rearrange("p c e -> p (c e)"))
            bb_sb = wp.tile([128, D], F32)
            nc.sync.dma_start(bb_sb[:], bb[:])

            emit_xtb_load(0)
            ones0 = Vs[0][:].rearrange("p (c o) -> p c o", o=65)[:, :, 64:65]
            nc.gpsimd.memset(ones0, 1.0)
            ones1 = Vs[1][:].rearrange("p (c o) -> p c o", o=65)[:, :, 64:65]
            nc.gpsimd.memset(ones1, 1.0)
            emit_qk_fast(0)
            emit_v(0, range(N_KC))
            emit_xtb_load(1)

            # batch-0 attention, batch-1 projection interleaved
            for t in range(N_QT):
                att_unit(0, 0, t)
                emit_qk_slice(1, t)
            for t in range(N_QT):
                att_unit(0, 1, t)
                emit_v(1, range(4 * t, 4 * t + 4))
            emit_a2a(0)
            emit_x2_loads(0)

            # batch-1 attention, batch-0 out-projection interleaved
            pieces = [(st, et) for st in range(S_SLICE // 128)
                      for et in range(D // 512)]
            for t in range(N_QT):
                att_unit(1, 0, t)
                st, et = pieces[t]
                emit_outproj_piece(0, st, et, wot_sb, bb_sb)
            for t in range(N_QT):
                att_unit(1, 1, t)
            emit_a2a(1)
            emit_x2_loads(1)
            for st, et in pieces:
                emit_outproj_piece(1, st, et, wot_sb, bb_sb)

    nc.compile()
    return nc


def _prep_chunked(a_t):
    """[Din, E] (already transposed) -> [128, Din//128, E] SBUF-chunk layout."""
    din, e = a_t.shape
    return np.ascontiguousarray(
        a_t.reshape(din // 128, 128, e).transpose(1, 0, 2)
    )


def kernel(x, w_qkv, w_out, b_out):
    global _compiled, last_results
    if _compiled is None:
        _compiled = _build()
    nc = _compiled

    x = np.asarray(x, dtype=np.float32)
    w_qkv = np.asarray(w_qkv, dtype=np.float32)
    w_out = np.asarray(w_out, dtype=np.float32)
    b_out = np.asarray(b_out, dtype=np.float32)

    # x^T in chunk layout: [B, 128, N_CH, S], bf16
    xt_full = x.transpose(0, 2, 1)  # [B, D, S]
    xtb_prep = np.ascontiguousarray(
        xt_full.reshape(B, N_CH, 128, S).transpose(0, 2, 1, 3)
    ).astype(ml_dtypes.bfloat16)

    wot_prep = _prep_chunked(np.ascontiguousarray(w_out.T)).astype(ml_dtypes.bfloat16)
    bb_np = np.ascontiguousarray(np.broadcast_to(b_out, (128, D)))

    in_maps = []
    for c in range(N_CORES):
        hA, hB = HEADS_PER_CORE * c, HEADS_PER_CORE * c + 1
        rows = np.r_[hA * DH:(hA + 1) * DH, hB * DH:(hB + 1) * DH]
        wq = w_qkv[rows, :]               # [128, D]
        wk = w_qkv[D + rows, :]
        wv = w_qkv[2 * D + rows, :]
        in_maps.append({
            "xtb": xtb_prep,
            "wqt": _prep_chunked(np.ascontiguousarray(wq.T)).astype(ml_dtypes.bfloat16),
            "wkt": _prep_chunked(np.ascontiguousarray(wk.T)).astype(ml_dtypes.bfloat16),
            "wvt": _prep_chunked(np.ascontiguousarray(wv.T)).astype(ml_dtypes.bfloat16),
            "wot": wot_prep,
            "bb": bb_np,
        })

    last_results = bass_utils.run_bass_kernel_spmd(
        nc, in_maps, core_ids=list(range(N_CORES))
    )
    out = np.concatenate(
        [last_results.results[c]["oc"] for c in range(N_CORES)], axis=1
    )
    return out


# revision 23
# speedup vs baseline: 1.0513x; 1.0513x over previous
"""Multi-head attention (B=2, S=2048, D=1024, H=16) on 8 Trainium2 cores.

Sharding: tensor-parallel over heads for QKV+attention (2 heads/core),
then an AllToAll reshards the attention output so each core computes the
output projection for its own 256-row slice of the sequence (both batches).
Host assembles the full output by concatenating the per-core slices.

Device pipeline per core (SPMD, identical program, shard-specific data):
  - starting-gun AllGather to absorb inter-core launch skew
  - PE warmup matmuls while x^T streams in (HAM clock ramp)
  - QKV projection (bf16): batch 0 in a ch-outer fast-start variant,
    batch 1 interleaved into batch-0 attention emission (PE executes its
    instruction stream in order)
  - attention: S^T = K^T.T @ Q^T, exp on ACT (PSUM [128,1024]), A*V with a
    ones column on V producing the softmax denominator (M=65), normalize
    via reciprocal on a [64,8] reshape + DMA partition-broadcast
  - AllToAll (bf16) per batch, out-projection overlapped.

PSUM budget (8 banks): tag s [128,1024]x2 = 4, tag av [65,512]x3 = 3,
tag o [128,512]x1 = 1. QK proj borrows s slots, V/warmup/outproj borrow o.
"""
import os
import sys

sys.path.insert(0, "/opt/trn_rl_repo")

import numpy as np
import ml_dtypes

import concourse.bass as bass
import concourse.tile as tile
from concourse import bacc, mybir
from concourse import bass_utils

B = 2
S = 2048
D = 1024
H = 16
DH = 64
N_CORES = 8
HEADS_PER_CORE = H // N_CORES          # 2
S_SLICE = S // N_CORES                 # 256
N_CH = D // 128                        # 8 contraction chunks
N_QT = S // 512                        # 4 q tiles
N_KC = S // 128                        # 16 k chunks

F32 = mybir.dt.float32
BF16 = mybir.dt.bfloat16

_compiled = None
last_results = None


def _build():
    nc = bacc.Bacc(
        "TRN2",
        target_bir_lowering=False,
        debug=False,
        enable_asserts=True,
        num_devices=N_CORES,
    )

    xtb = nc.dram_tensor("xtb", [B, 128, N_CH, S], BF16, kind="ExternalInput").ap()
    wqt = nc.dram_tensor("wqt", [128, N_CH, 128], BF16, kind="ExternalInput").ap()
    wkt = nc.dram_tensor("wkt", [128, N_CH, 128], BF16, kind="ExternalInput").ap()
    wvt = nc.dram_tensor("wvt", [128, N_CH, 128], BF16, kind="ExternalInput").ap()
    wot = nc.dram_tensor("wot", [128, N_CH, D], BF16, kind="ExternalInput").ap()
    bb = nc.dram_tensor("bb", [128, D], F32, kind="ExternalInput").ap()
    oc = nc.dram_tensor("oc", [B, S_SLICE, D], F32, kind="ExternalOutput").ap()

    EXP = mybir.ActivationFunctionType.Exp
    SCALE = DH ** -0.5

    with tile.TileContext(nc) as tc:
        with (
            tc.tile_pool(name="w", bufs=1) as wp,
            tc.tile_pool(name="qkt", bufs=1) as qktp,
            tc.tile_pool(name="vsb", bufs=1) as vsbp,
            tc.tile_pool(name="xtb", bufs=2) as xtbp,
            tc.tile_pool(name="pt", bufs=2) as ptp,
            tc.tile_pool(name="norm", bufs=2) as normp,
            tc.tile_pool(name="x2", bufs=1) as x2p,
            tc.tile_pool(name="outsb", bufs=2) as outp,
            tc.tile_pool(name="dram", bufs=1, space="DRAM") as dram,
            tc.tile_pool(name="dramsc", bufs=4, space="DRAM") as dramsc,
            tc.tile_pool(name="sps", bufs=2, space="PSUM") as sps,
            tc.tile_pool(name="avps", bufs=3, space="PSUM") as avps,
            tc.tile_pool(name="ops", bufs=1, space="PSUM") as ops,
        ):
            # ---- weights ----
            wqt_sb = wp.tile([128, N_CH * 128], BF16)
            nc.sync.dma_start(wqt_sb[:], wqt[:].rearrange("p c e -> p (c e)"))
            wkt_sb = wp.tile([128, N_CH * 128], BF16)
            nc.sync.dma_start(wkt_sb[:], wkt[:].rearrange("p c e -> p (c e)"))
            wvt_sb = wp.tile([128, N_CH * 128], BF16)
            nc.sync.dma_start(wvt_sb[:], wvt[:].rearrange("p c e -> p (c e)"))

            # ---- starting gun: tiny AllGather aligns the 8 cores ----
            gun_in = dram.tile([1, 16], F32, name="gun_in")
            gun_out = dram.tile([N_CORES, 16], F32, name="gun_out")
            gun_sb = wp.tile([1, 16], F32)
            nc.gpsimd.memset(gun_sb[:], 0.0)
            nc.sync.dma_start(gun_in[:], gun_sb[:])
            nc.gpsimd.collective_compute(
                "AllGather", mybir.AluOpType.bypass,
                replica_groups=[list(range(N_CORES))],
                ins=[gun_in[:]], outs=[gun_out[:]],
            )

            # ---- PE warmup while DMAs stream ----
            warm = wp.tile([128, 512], BF16)
            nc.gpsimd.memset(warm[:], 0.0)
            for i in range(32):
                wps = ops.tile([128, 512], F32, tag="o", name="wps")
                nc.tensor.matmul(wps[:], lhsT=warm[:, 0:128], rhs=warm[:],
                                 start=True, stop=True)

            xtb_sbs = [None, None]
            Qt, Kt, Vs = [], [], []
            for b in range(B):
                Qt.append(qktp.tile([128, S], BF16, tag=f"qt{b}", name=f"qt{b}"))
                Kt.append(qktp.tile([128, S], BF16, tag=f"kt{b}", name=f"kt{b}"))
                Vs.append(vsbp.tile([128, N_KC * 130], BF16, tag=f"v{b}",
                                    name=f"v{b}"))

            def emit_xtb_load(b):
                t_ = xtbp.tile([128, N_CH * S], BF16, tag="xtb", name="xtb_sb")
                xtb_sbs[b] = t_
                for ch in range(N_CH):
                    nc.sync.dma_start(t_[:, ch * S:(ch + 1) * S], xtb[b, :, ch, :])

            def emit_qk_fast(b):
                """ch-outer, 2 passes of 2 t-tiles; starts as chunks land."""
                for p_ in range(2):
                    q_ps = sps.tile([128, 1024], F32, tag="s", name="q_ps")
                    k_ps = sps.tile([128, 1024], F32, tag="s", name="k_ps")
                    for ch in range(N_CH):
                        for j in range(2):
                            t = 2 * p_ + j
                            rhs = xtb_sbs[b][:, ch * S + t * 512:
                                             ch * S + (t + 1) * 512]
                            nc.tensor.matmul(
                                q_ps[:, j * 512:(j + 1) * 512],
                                lhsT=wqt_sb[:, ch * 128:(ch + 1) * 128],
                                rhs=rhs, start=(ch == 0), stop=(ch == N_CH - 1),
                            )
                        for j in range(2):
                            t = 2 * p_ + j
                            rhs = xtb_sbs[b][:, ch * S + t * 512:
                                             ch * S + (t + 1) * 512]
                            nc.tensor.matmul(
                                k_ps[:, j * 512:(j + 1) * 512],
                                lhsT=wkt_sb[:, ch * 128:(ch + 1) * 128],
                                rhs=rhs, start=(ch == 0), stop=(ch == N_CH - 1),
                            )
                    nc.vector.tensor_copy(
                        Qt[b][:, p_ * 1024:(p_ + 1) * 1024], q_ps[:])
                    nc.vector.tensor_copy(
                        Kt[b][:, p_ * 1024:(p_ + 1) * 1024], k_ps[:])

            def emit_qk_slice(b, t):
                """ch-inner, one psum at a time (attention running)."""
                for which, w_sb, dst in (("q", wqt_sb, Qt[b]), ("k", wkt_sb, Kt[b])):
                    ps_ = sps.tile([128, 1024], F32, tag="s", name=f"{which}_ps1")
                    for ch in range(N_CH):
                        nc.tensor.matmul(
                            ps_[:, 0:512],
                            lhsT=w_sb[:, ch * 128:(ch + 1) * 128],
                            rhs=xtb_sbs[b][:, ch * S + t * 512:
                                           ch * S + (t + 1) * 512],
                            start=(ch == 0), stop=(ch == N_CH - 1),
                        )
                    nc.vector.tensor_copy(
                        dst[:, t * 512:(t + 1) * 512], ps_[:, 0:512])

            def emit_v(b, sts):
                v_sb = Vs[b]
                for st in sts:
                    v_ps = ops.tile([128, 512], F32, tag="o", name="v_ps")
                    for ch in range(N_CH):
                        nc.tensor.matmul(
                            v_ps[:, 0:128],
                            lhsT=xtb_sbs[b][:, ch * S + st * 128:
                                            ch * S + (st + 1) * 128],
                            rhs=wvt_sb[:, ch * 128:(ch + 1) * 128],
                            start=(ch == 0), stop=(ch == N_CH - 1),
                        )
                    dst = v_sb[:].rearrange("p (c o) -> p c o", o=65)[
                        :, 2 * st:2 * st + 2, 0:64
                    ]
                    nc.vector.tensor_copy(
                        dst, v_ps[:, 0:128].rearrange("p (h e) -> p h e", e=64)
                    )

            a2a_in = [[dram.tile([N_CORES, 64, S_SLICE], BF16,
                                 tag=f"a2ai{b}{h}", name=f"a2ai{b}{h}")
                       for h in range(2)] for b in range(B)]
            a2a_out = [[dram.tile([N_CORES, 64, S_SLICE], BF16,
                                  tag=f"a2ao{b}{h}", name=f"a2ao{b}{h}")
                        for h in range(2)] for b in range(B)]

            def att_unit(b, h, t, v_interleave=None):
                hp = slice(h * 64, (h + 1) * 64)
                voff = h * 65
                av = avps.tile([65, 512], F32, tag="av", name="av")
                qs = slice(t * 512, (t + 1) * 512)
                for cc in range(N_KC // 2):
                    if v_interleave is not None:
                        emit_v(v_interleave, (2 * cc, 2 * cc + 1))
                    s_ps = sps.tile([128, 1024], F32, tag="s", name="s_ps")
                    for j in range(2):
                        c = 2 * cc + j
                        ks = slice(c * 128, (c + 1) * 128)
                        nc.tensor.matmul(
                            s_ps[:, j * 512:(j + 1) * 512],
                            lhsT=Kt[b][hp, ks], rhs=Qt[b][hp, qs],
                            start=True, stop=True,
                        )
                    p_sb = ptp.tile([128, 1024], BF16, tag="p", name="p_sb")
                    nc.scalar.activation(p_sb[:], s_ps[:], EXP, scale=SCALE)
                    for j in range(2):
                        c = 2 * cc + j
                        nc.tensor.matmul(
                            av[:],
                            lhsT=Vs[b][:, c * 130 + voff:c * 130 + voff + 65],
                            rhs=p_sb[:, j * 512:(j + 1) * 512],
                            start=(c == 0), stop=(c == N_KC - 1),
                            skip_group_check=True,
                        )
                # normalize: denom -> [64,8] reshape -> reciprocal -> broadcast
                den_sb = normp.tile([1, 512], F32, tag="dsb", name="den_sb")
                nc.vector.tensor_copy(den_sb[:], av[64:65, :])
                den_d = dramsc.tile([512], F32, tag="dend", name="den_d")
                nc.sync.dma_start(
                    den_d[:].rearrange("(a q) -> a q", a=1), den_sb[:])
                den64 = normp.tile([64, 8], F32, tag="d64", name="den64")
                nc.sync.dma_start(
                    den64[:], den_d[:].rearrange("(p q) -> p q", p=64))
                rec64 = normp.tile([64, 8], F32, tag="r64", name="rec64")
                nc.vector.reciprocal(rec64[:], den64[:])
                rsc = dramsc.tile([512], F32, tag="rsc", name="rsc")
                nc.sync.dma_start(
                    rsc[:].rearrange("(p q) -> p q", p=64), rec64[:])
                bcast = normp.tile([64, 512], F32, tag="bc", name="bcast")
                nc.sync.dma_start(
                    bcast[:],
                    rsc[:].rearrange("(a q) -> a q", a=1).broadcast_to([64, 512]),
                )
                o_sb = normp.tile([64, 512], BF16, tag="ob", name="o_sb")
                nc.vector.tensor_mul(o_sb[:], av[0:64, :], bcast[:])
                for j in range(2):
                    nc.sync.dma_start(
                        a2a_in[b][h][2 * t + j, :, :],
                        o_sb[:, j * S_SLICE:(j + 1) * S_SLICE],
                    )

            def emit_a2a(b, h):
                nc.gpsimd.collective_compute(
                    "AllToAll", mybir.AluOpType.bypass,
                    replica_groups=[list(range(N_CORES))],
                    ins=[a2a_in[b][h][:]], outs=[a2a_out[b][h][:]],
                )

            x2_tiles = {}

            def emit_x2_loads(b):
                x2 = []
                for i in range(N_CH):
                    x2_sb = x2p.tile([128, S_SLICE], BF16, tag=f"x2_{b}_{i}",
                                     name=f"x2_{b}_{i}")
                    nc.sync.dma_start(x2_sb[0:64, :], a2a_out[b][0][i])
                    nc.sync.dma_start(x2_sb[64:128, :], a2a_out[b][1][i])
                    x2.append(x2_sb)
                x2_tiles[b] = x2

            def emit_outproj_piece(b, st, et, wot_sb, bb_sb):
                o_ps = ops.tile([128, 512], F32, tag="o", name="o_ps")
                for ch in range(N_CH):
                    nc.tensor.matmul(
                        o_ps[:],
                        lhsT=x2_tiles[b][ch][:, st * 128:(st + 1) * 128],
                        rhs=wot_sb[:, ch * D + et * 512:ch * D + (et + 1) * 512],
                        start=(ch == 0), stop=(ch == N_CH - 1),
                    )
                out_sb = outp.tile([128, 512], F32, tag="osb", name="out_sb")
                nc.vector.tensor_add(
                    out_sb[:], o_ps[:], bb_sb[:, et * 512:(et + 1) * 512])
                nc.sync.dma_start(
                    oc[b, st * 128:(st + 1) * 128, et * 512:(et + 1) * 512],
                    out_sb[:],
                )

            # ================= pipeline =================
            emit_xtb_load(0)
            ones0 = Vs[0][:].rearrange("p (c o) -> p c o", o=65)[:, :, 64:65]
            nc.gpsimd.memset(ones0, 1.0)
            ones1 = Vs[1][:].rearrange("p (c o) -> p c o", o=65)[:, :, 64:65]
            nc.gpsimd.memset(ones1, 1.0)
            emit_qk_fast(0)
            emit_v(0, range(N_KC))
            emit_xtb_load(1)

            # batch-0 attention, batch-1 projection interleaved
            for t in range(N_QT):
                att_unit(0, 0, t)
                emit_qk_slice(1, t)
            emit_a2a(0, 0)
            for t in range(N_QT):
                att_unit(0, 1, t)
                emit_v(1, range(4 * t, 4 * t + 4))
            emit_a2a(0, 1)
            emit_x2_loads(0)
            wot_sb = wp.tile([128, N_CH * D], BF16)
            nc.sync.dma_start(wot_sb[:], wot[:].rearrange("p c e -> p (c e)"))
            bb_sb = wp.tile([128, D], F32)
            nc.sync.dma_start(bb_sb[:], bb[:])

            # batch-1 attention, batch-0 out-projection interleaved
            pieces = [(st, et) for st in range(S_SLICE // 128)
                      for et in range(D // 512)]
            for t in range(N_QT):
                att_unit(1, 0, t)
                st, et = pieces[t]
                emit_outproj_piece(0, st, et, wot_sb, bb_sb)
            emit_a2a(1, 0)
            for t in range(N_QT):
                att_unit(1, 1, t)
            emit_a2a(1, 1)
            emit_x2_loads(1)
            for st, et in pieces:
                emit_outproj_piece(1, st, et, wot_sb, bb_sb)

    nc.compile()
    return nc


def _prep_chunked(a_t):
    """[Din, E] (already transposed) -> [128, Din//128, E] SBUF-chunk layout."""
    din, e = a_t.shape
    return np.ascontiguousarray(
        a_t.reshape(din // 128, 128, e).transpose(1, 0, 2)
    )


def kernel(x, w_qkv, w_out, b_out):
    global _compiled, last_results
    if _compiled is None:
        _compiled = _build()
    nc = _compiled

    x = np.asarray(x, dtype=np.float32)
    w_qkv = np.asarray(w_qkv, dtype=np.float32)
    w_out = np.asarray(w_out, dtype=np.float32)
    b_out = np.asarray(b_out, dtype=np.float32)

    # x^T in chunk layout: [B, 128, N_CH, S], bf16
    xt_full = x.transpose(0, 2, 1)  # [B, D, S]
    xtb_prep = np.ascontiguousarray(
        xt_full.reshape(B, N_CH, 128, S).transpose(0, 2, 1, 3)
    ).astype(ml_dtypes.bfloat16)

    wot_prep = _prep_chunked(np.ascontiguousarray(w_out.T)).astype(ml_dtypes.bfloat16)
    bb_np = np.ascontiguousarray(np.broadcast_to(b_out, (128, D)))

    in_maps = []
    for c in range(N_CORES):
        hA, hB = HEADS_PER_CORE * c, HEADS_PER_CORE * c + 1
        rows = np.r_[hA * DH:(hA + 1) * DH, hB * DH:(hB + 1) * DH]
        wq = w_qkv[rows, :]               # [128, D]
        wk = w_qkv[D + rows, :]
        wv = w_qkv[2 * D + rows, :]
        in_maps.append({
            "xtb": xtb_prep,
            "wqt": _prep_chunked(np.ascontiguousarray(wq.T)).astype(ml_dtypes.bfloat16),
            "wkt": _prep_chunked(np.ascontiguousarray(wk.T)).astype(ml_dtypes.bfloat16),
            "wvt": _prep_chunked(np.ascontiguousarray(wv.T)).astype(ml_dtypes.bfloat16),
            "wot": wot_prep,
            "bb": bb_np,
        })

    last_results = bass_utils.run_bass_kernel_spmd(
        nc, in_maps, core_ids=list(range(N_CORES))
    )
    out = np.concatenate(
        [last_results.results[c]["oc"] for c in range(N_CORES)], axis=1
    )
    return out


# revision 24
# speedup vs baseline: 1.0874x; 1.0343x over previous
"""Multi-head attention (B=2, S=2048, D=1024, H=16) on 8 Trainium2 cores.

Sharding: tensor-parallel over heads for QKV+attention (2 heads/core),
then an AllToAll reshards the attention output so each core computes the
output projection for its own 256-row slice of the sequence (both batches).
Host assembles the full output by concatenating the per-core slices.

Device pipeline per core (SPMD, identical program, shard-specific data):
  - starting-gun AllGather to absorb inter-core launch skew
  - PE warmup matmuls while x^T streams in (HAM clock ramp)
  - QKV projection (bf16): batch 0 in a ch-outer fast-start variant,
    batch 1 interleaved into batch-0 attention emission (PE executes its
    instruction stream in order)
  - attention: S^T = K^T.T @ Q^T, exp on ACT (PSUM [128,1024]), A*V with a
    ones column on V producing the softmax denominator (M=65), normalize
    via reciprocal on a [64,8] reshape + DMA partition-broadcast
  - AllToAll (bf16) per batch, out-projection overlapped.

PSUM budget (8 banks): tag s [128,1024]x2 = 4, tag av [65,512]x3 = 3,
tag o [128,512]x1 = 1. QK proj borrows s slots, V/warmup/outproj borrow o.
"""
import os
import sys

sys.path.insert(0, "/opt/trn_rl_repo")

import numpy as np
import ml_dtypes

import concourse.bass as bass
import concourse.tile as tile
from concourse import bacc, mybir
from concourse import bass_utils

B = 2
S = 2048
D = 1024
H = 16
DH = 64
N_CORES = 8
HEADS_PER_CORE = H // N_CORES          # 2
S_SLICE = S // N_CORES                 # 256
N_CH = D // 128                        # 8 contraction chunks
N_QT = S // 512                        # 4 q tiles
N_KC = S // 128                        # 16 k chunks

F32 = mybir.dt.float32
BF16 = mybir.dt.bfloat16

_compiled = None
last_results = None


def _build():
    nc = bacc.Bacc(
        "TRN2",
        target_bir_lowering=False,
        debug=False,
        enable_asserts=True,
        num_devices=N_CORES,
    )

    xtb = nc.dram_tensor("xtb", [B, 128, N_CH, S], BF16, kind="ExternalInput").ap()
    wqt = nc.dram_tensor("wqt", [128, N_CH, 128], BF16, kind="ExternalInput").ap()
    wkt = nc.dram_tensor("wkt", [128, N_CH, 128], BF16, kind="ExternalInput").ap()
    wvt = nc.dram_tensor("wvt", [128, N_CH, 128], BF16, kind="ExternalInput").ap()
    wot = nc.dram_tensor("wot", [128, N_CH, D], BF16, kind="ExternalInput").ap()
    bb = nc.dram_tensor("bb", [128, D], F32, kind="ExternalInput").ap()
    oc = nc.dram_tensor("oc", [B, S_SLICE, D], F32, kind="ExternalOutput").ap()

    EXP = mybir.ActivationFunctionType.Exp
    SCALE = DH ** -0.5

    with tile.TileContext(nc) as tc:
        with (
            tc.tile_pool(name="w", bufs=1) as wp,
            tc.tile_pool(name="qkt", bufs=1) as qktp,
            tc.tile_pool(name="vsb", bufs=1) as vsbp,
            tc.tile_pool(name="xtb", bufs=2) as xtbp,
            tc.tile_pool(name="pt", bufs=2) as ptp,
            tc.tile_pool(name="norm", bufs=2) as normp,
            tc.tile_pool(name="x2", bufs=1) as x2p,
            tc.tile_pool(name="outsb", bufs=2) as outp,
            tc.tile_pool(name="dram", bufs=1, space="DRAM") as dram,
            tc.tile_pool(name="dramsc", bufs=4, space="DRAM") as dramsc,
            tc.tile_pool(name="sps", bufs=2, space="PSUM") as sps,
            tc.tile_pool(name="avps", bufs=3, space="PSUM") as avps,
            tc.tile_pool(name="ops", bufs=1, space="PSUM") as ops,
        ):
            # ---- weights ----
            wqt_sb = wp.tile([128, N_CH * 128], BF16)
            nc.sync.dma_start(wqt_sb[:], wqt[:].rearrange("p c e -> p (c e)"))
            wkt_sb = wp.tile([128, N_CH * 128], BF16)
            nc.sync.dma_start(wkt_sb[:], wkt[:].rearrange("p c e -> p (c e)"))
            wvt_sb = wp.tile([128, N_CH * 128], BF16)
            nc.sync.dma_start(wvt_sb[:], wvt[:].rearrange("p c e -> p (c e)"))

            # ---- starting gun: tiny AllGather aligns the 8 cores ----
            gun_in = dram.tile([1, 16], F32, name="gun_in")
            gun_out = dram.tile([N_CORES, 16], F32, name="gun_out")
            gun_sb = wp.tile([1, 16], F32)
            nc.gpsimd.memset(gun_sb[:], 0.0)
            nc.sync.dma_start(gun_in[:], gun_sb[:])
            nc.gpsimd.collective_compute(
                "AllGather", mybir.AluOpType.bypass,
                replica_groups=[list(range(N_CORES))],
                ins=[gun_in[:]], outs=[gun_out[:]],
            )

            # ---- PE warmup while DMAs stream ----
            warm = wp.tile([128, 512], BF16)
            nc.gpsimd.memset(warm[:], 0.0)
            for i in range(32):
                wps = ops.tile([128, 512], F32, tag="o", name="wps")
                nc.tensor.matmul(wps[:], lhsT=warm[:, 0:128], rhs=warm[:],
                                 start=True, stop=True)

            xtb_sbs = [None, None]
            Qt, Kt, Vs = [], [], []
            for b in range(B):
                Qt.append(qktp.tile([128, S], BF16, tag=f"qt{b}", name=f"qt{b}"))
                Kt.append(qktp.tile([128, S], BF16, tag=f"kt{b}", name=f"kt{b}"))
                Vs.append(vsbp.tile([128, N_KC * 130], BF16, tag=f"v{b}",
                                    name=f"v{b}"))

            def emit_xtb_load(b):
                t_ = xtbp.tile([128, N_CH * S], BF16, tag="xtb", name="xtb_sb")
                xtb_sbs[b] = t_
                for ch in range(N_CH):
                    nc.sync.dma_start(t_[:, ch * S:(ch + 1) * S], xtb[b, :, ch, :])

            def emit_qk_fast(b):
                """ch-outer, 2 passes of 2 t-tiles; starts as chunks land."""
                for p_ in range(2):
                    q_ps = sps.tile([128, 1024], F32, tag="s", name="q_ps")
                    k_ps = sps.tile([128, 1024], F32, tag="s", name="k_ps")
                    for ch in range(N_CH):
                        for j in range(2):
                            t = 2 * p_ + j
                            rhs = xtb_sbs[b][:, ch * S + t * 512:
                                             ch * S + (t + 1) * 512]
                            nc.tensor.matmul(
                                q_ps[:, j * 512:(j + 1) * 512],
                                lhsT=wqt_sb[:, ch * 128:(ch + 1) * 128],
                                rhs=rhs, start=(ch == 0), stop=(ch == N_CH - 1),
                            )
                        for j in range(2):
                            t = 2 * p_ + j
                            rhs = xtb_sbs[b][:, ch * S + t * 512:
                                             ch * S + (t + 1) * 512]
                            nc.tensor.matmul(
                                k_ps[:, j * 512:(j + 1) * 512],
                                lhsT=wkt_sb[:, ch * 128:(ch + 1) * 128],
                                rhs=rhs, start=(ch == 0), stop=(ch == N_CH - 1),
                            )
                    nc.vector.tensor_copy(
                        Qt[b][:, p_ * 1024:(p_ + 1) * 1024], q_ps[:])
                    nc.vector.tensor_copy(
                        Kt[b][:, p_ * 1024:(p_ + 1) * 1024], k_ps[:])

            def emit_qk_slice(b, t):
                """ch-inner, one psum at a time (attention running)."""
                for which, w_sb, dst in (("q", wqt_sb, Qt[b]), ("k", wkt_sb, Kt[b])):
                    ps_ = sps.tile([128, 1024], F32, tag="s", name=f"{which}_ps1")
                    for ch in range(N_CH):
                        nc.tensor.matmul(
                            ps_[:, 0:512],
                            lhsT=w_sb[:, ch * 128:(ch + 1) * 128],
                            rhs=xtb_sbs[b][:, ch * S + t * 512:
                                           ch * S + (t + 1) * 512],
                            start=(ch == 0), stop=(ch == N_CH - 1),
                        )
                    nc.vector.tensor_copy(
                        dst[:, t * 512:(t + 1) * 512], ps_[:, 0:512])

            def emit_v(b, sts):
                v_sb = Vs[b]
                for st in sts:
                    v_ps = ops.tile([128, 512], F32, tag="o", name="v_ps")
                    for ch in range(N_CH):
                        nc.tensor.matmul(
                            v_ps[:, 0:128],
                            lhsT=xtb_sbs[b][:, ch * S + st * 128:
                                            ch * S + (st + 1) * 128],
                            rhs=wvt_sb[:, ch * 128:(ch + 1) * 128],
                            start=(ch == 0), stop=(ch == N_CH - 1),
                        )
                    dst = v_sb[:].rearrange("p (c o) -> p c o", o=65)[
                        :, 2 * st:2 * st + 2, 0:64
                    ]
                    nc.vector.tensor_copy(
                        dst, v_ps[:, 0:128].rearrange("p (h e) -> p h e", e=64)
                    )

            a2a_in = [dram.tile([N_CORES, 128, S_SLICE], BF16, tag=f"a2ai{b}",
                                name=f"a2ai{b}") for b in range(B)]
            a2a_out = [dram.tile([N_CORES, 128, S_SLICE], BF16, tag=f"a2ao{b}",
                                 name=f"a2ao{b}") for b in range(B)]

            def att_unit(b, h, t, v_interleave=None):
                hp = slice(h * 64, (h + 1) * 64)
                voff = h * 65
                av = avps.tile([65, 512], F32, tag="av", name="av")
                qs = slice(t * 512, (t + 1) * 512)
                for cc in range(N_KC // 2):
                    if v_interleave is not None:
                        emit_v(v_interleave, (2 * cc, 2 * cc + 1))
                    s_ps = sps.tile([128, 1024], F32, tag="s", name="s_ps")
                    for j in range(2):
                        c = 2 * cc + j
                        ks = slice(c * 128, (c + 1) * 128)
                        nc.tensor.matmul(
                            s_ps[:, j * 512:(j + 1) * 512],
                            lhsT=Kt[b][hp, ks], rhs=Qt[b][hp, qs],
                            start=True, stop=True,
                        )
                    p_sb = ptp.tile([128, 1024], BF16, tag="p", name="p_sb")
                    nc.scalar.activation(p_sb[:], s_ps[:], EXP, scale=SCALE)
                    for j in range(2):
                        c = 2 * cc + j
                        nc.tensor.matmul(
                            av[:],
                            lhsT=Vs[b][:, c * 130 + voff:c * 130 + voff + 65],
                            rhs=p_sb[:, j * 512:(j + 1) * 512],
                            start=(c == 0), stop=(c == N_KC - 1),
                            skip_group_check=True,
                        )
                # normalize: denom -> [64,8] reshape -> reciprocal -> broadcast
                den_sb = normp.tile([1, 512], F32, tag="dsb", name="den_sb")
                nc.vector.tensor_copy(den_sb[:], av[64:65, :])
                den_d = dramsc.tile([512], F32, tag="dend", name="den_d")
                nc.sync.dma_start(
                    den_d[:].rearrange("(a q) -> a q", a=1), den_sb[:])
                den64 = normp.tile([64, 8], F32, tag="d64", name="den64")
                nc.sync.dma_start(
                    den64[:], den_d[:].rearrange("(p q) -> p q", p=64))
                rec64 = normp.tile([64, 8], F32, tag="r64", name="rec64")
                nc.vector.reciprocal(rec64[:], den64[:])
                rsc = dramsc.tile([512], F32, tag="rsc", name="rsc")
                nc.sync.dma_start(
                    rsc[:].rearrange("(p q) -> p q", p=64), rec64[:])
                bcast = normp.tile([64, 512], F32, tag="bc", name="bcast")
                nc.sync.dma_start(
                    bcast[:],
                    rsc[:].rearrange("(a q) -> a q", a=1).broadcast_to([64, 512]),
                )
                o_sb = normp.tile([64, 512], BF16, tag="ob", name="o_sb")
                nc.vector.tensor_mul(o_sb[:], av[0:64, :], bcast[:])
                for j in range(2):
                    nc.sync.dma_start(
                        a2a_in[b][2 * t + j, hp, :],
                        o_sb[:, j * S_SLICE:(j + 1) * S_SLICE],
                    )

            def emit_a2a(b):
                nc.gpsimd.collective_compute(
                    "AllToAll", mybir.AluOpType.bypass,
                    replica_groups=[list(range(N_CORES))],
                    ins=[a2a_in[b][:]], outs=[a2a_out[b][:]],
                )

            x2_tiles = {}

            def emit_x2_loads(b):
                x2 = []
                for i in range(N_CH):
                    x2_sb = x2p.tile([128, S_SLICE], BF16, tag=f"x2_{b}_{i}",
                                     name=f"x2_{b}_{i}")
                    nc.sync.dma_start(x2_sb[:], a2a_out[b][i])
                    x2.append(x2_sb)
                x2_tiles[b] = x2

            def emit_outproj_piece(b, st, et, wot_sb, bb_sb):
                o_ps = ops.tile([128, 512], F32, tag="o", name="o_ps")
                for ch in range(N_CH):
                    nc.tensor.matmul(
                        o_ps[:],
                        lhsT=x2_tiles[b][ch][:, st * 128:(st + 1) * 128],
                        rhs=wot_sb[:, ch * D + et * 512:ch * D + (et + 1) * 512],
                        start=(ch == 0), stop=(ch == N_CH - 1),
                    )
                out_sb = outp.tile([128, 512], F32, tag="osb", name="out_sb")
                nc.vector.tensor_add(
                    out_sb[:], o_ps[:], bb_sb[:, et * 512:(et + 1) * 512])
                nc.sync.dma_start(
                    oc[b, st * 128:(st + 1) * 128, et * 512:(et + 1) * 512],
                    out_sb[:],
                )

            # ================= pipeline =================
            emit_xtb_load(0)
            ones0 = Vs[0][:].rearrange("p (c o) -> p c o", o=65)[:, :, 64:65]
            nc.gpsimd.memset(ones0, 1.0)
            ones1 = Vs[1][:].rearrange("p (c o) -> p c o", o=65)[:, :, 64:65]
            nc.gpsimd.memset(ones1, 1.0)
            emit_qk_fast(0)
            emit_v(0, range(N_KC))
            emit_xtb_load(1)

            # batch-0 attention, batch-1 projection interleaved
            for t in range(N_QT):
                att_unit(0, 0, t)
                emit_qk_slice(1, t)
            for t in range(N_QT):
                att_unit(0, 1, t)
                emit_v(1, range(4 * t, 4 * t + 4))
            emit_a2a(0)
            emit_x2_loads(0)
            wot_sb = wp.tile([128, N_CH * D], BF16)
            nc.sync.dma_start(wot_sb[:], wot[:].rearrange("p c e -> p (c e)"))
            bb_sb = wp.tile([128, D], F32)
            nc.sync.dma_start(bb_sb[:], bb[:])

            # batch-1 attention, batch-0 out-projection interleaved
            pieces = [(st, et) for st in range(S_SLICE // 128)
                      for et in range(D // 512)]
            for t in range(N_QT):
                att_unit(1, 0, t)
                st, et = pieces[t]
                emit_outproj_piece(0, st, et, wot_sb, bb_sb)
            for t in range(N_QT):
                att_unit(1, 1, t)
            emit_a2a(1)
            emit_x2_loads(1)
            for st, et in pieces:
                emit_outproj_piece(1, st, et, wot_sb, bb_sb)

    nc.compile()
    return nc


def _prep_chunked(a_t):
    """[Din, E] (already transposed) -> [128, Din//128, E] SBUF-chunk layout."""
    din, e = a_t.shape
    return np.ascontiguousarray(
        a_t.reshape(din // 128, 128, e).transpose(1, 0, 2)
    )


def kernel(x, w_qkv, w_out, b_out):
    global _compiled, last_results
    if _compiled is None:
        _compiled = _build()
    nc = _compiled

    x = np.asarray(x, dtype=np.float32)
    w_qkv = np.asarray(w_qkv, dtype=np.float32)
    w_out = np.asarray(w_out, dtype=np.float32)
    b_out = np.asarray(b_out, dtype=np.float32)

    # x^T in chunk layout: [B, 128, N_CH, S], bf16
    xt_full = x.transpose(0, 2, 1)  # [B, D, S]
    xtb_prep = np.ascontiguousarray(
        xt_full.reshape(B, N_CH, 128, S).transpose(0, 2, 1, 3)
    ).astype(ml_dtypes.bfloat16)

    wot_prep = _prep_chunked(np.ascontiguousarray(w_out.T)).astype(ml_dtypes.bfloat16)
    bb_np = np.ascontiguousarray(np.broadcast_to(b_out, (128, D)))

    in_maps = []
    for c in range(N_CORES):
        hA, hB = HEADS_PER_CORE * c, HEADS_PER_CORE * c + 1
        rows = np.r_[hA * DH:(hA + 1) * DH, hB * DH:(hB + 1) * DH]
        wq = w_qkv[rows, :]               # [128, D]
        wk = w_qkv[D + rows, :]
        wv = w_qkv[2 * D + rows, :]
        in_maps.append({
            "xtb": xtb_prep,
            "wqt": _prep_chunked(np.ascontiguousarray(wq.T)).astype(ml_dtypes.bfloat16),
            "wkt": _prep_chunked(np.ascontiguousarray(wk.T)).astype(ml_dtypes.bfloat16),
            "wvt": _prep_chunked(np.ascontiguousarray(wv.T)).astype(ml_dtypes.bfloat16),
            "wot": wot_prep,
            "bb": bb_np,
        })

    last_results = bass_utils.run_bass_kernel_spmd(
        nc, in_maps, core_ids=list(range(N_CORES))
    )
    out = np.concatenate(
        [last_results.results[c]["oc"] for c in range(N_CORES)], axis=1
    )
    return out
